# revision 6
# baseline (speedup 1.0000x reference)
"""Trainium2 kernel for nn_ConservationOfFeatureSimilarity.

Math (see reference): with xn = row-normalized feature embeddings (M, 256) and
zn = row-normalized frozen embeddings (M, 768), M = B*N = 3136:

  feat_sim  = xn @ xn.T        (M, M)
  frozen_sim= zn @ zn.T        (M, M)
  ranking   = triu+ * (feat-frozen) * [cls_i != cls_j] * [pidx_i == pidx_j] * mps_i*mps_j
  top5      = top_k(ranking.flat, 5);  sel rows/cols
  out       = mean |feat_sim[sel] - frozen_sim[sel]|  over (5, 2, M)
            = (sum over the 10 selected row indices of S[r]) / (10*M)
  where S_i = sum_j |feat_sim[i,j] - frozen_sim[i,j]|.

Device (8 NeuronCores): the dense O(M^2 * K) part — S row sums — runs as fp8
(e4m3) DoubleRow matmuls: embeddings are scaled by 16, quantized to fp8, and
diff tiles accumulate in PSUM over 4 DoubleRow groups (256-deep contraction
each; frozen chunks sign-flipped on the weights side so one gram matrix is
subtracted). |diff| is symmetric, so only blocks at-or-right-of the diagonal
of a (25 row-tile x 13 col-block) grid are computed: rows padded to
3200 = 25*128, cols to 3328 = 13*256. Core c owns row tiles {8t+c: t=0..3};
slot t computes col blocks c >= 4t (28 blocks/core; 13+9+5+1). Each block
contributes DVE row sums, and strictly-upper blocks contribute mirrored
column sums via a masked ones-matmul (the per-core 0/1 mask is data, not
code). The host drops the few below-diagonal row-sum partials.

Host: normalization/transposes, fp8 quantization, prototype argmax, the top-5
search (ranking is nonzero only for same-argmax-prototype pairs: ~25K of the
9.8M pairs, so it is evaluated sparsely in numpy), and the final combine.
"""

import sys

if "/opt/trn_rl_repo" not in sys.path:
    sys.path.insert(0, "/opt/trn_rl_repo")

import numpy as np
import ml_dtypes

BF16 = ml_dtypes.bfloat16
F8 = ml_dtypes.float8_e4m3

B, N, D, NF, P = 16, 196, 768, 256, 200
M = B * N                      # 3136
NCORES = 8
RT = 128                       # row tile height
NRT = 25                       # row tiles (rows padded to 3200)
MR = RT * NRT                  # 3200
CBW = 256                      # col block width
NCB = 13                       # col blocks (cols padded to 3328)
MC = CBW * NCB                 # 3328
NK = 8                         # 128-deep K chunks: 2 feat + 6 frozen
NG = 4                         # DoubleRow groups (256-deep each)
NSLOT = 4                      # row tiles per core (slot t -> tile 8t+c)
CSTART = (0, 4, 8, 12)         # first col block computed by slot t
SCALE = 16.0                   # fp8 quantization scale
K_ = 5
GAMMA = 1.0
EPS = 1e-8

# program-order block list: col-major so compute overlaps the col-band DMAs
BLOCKS = [(t, c) for c in range(NCB) for t in range(NSLOT) if c >= CSTART[t]]
NB = len(BLOCKS)               # 28
NMIR = sum(1 for t, c in BLOCKS if c > CSTART[t])  # 24 mirrored blocks

_COMPILED = None
_last_bass_results = None


def _build():
    from concourse import bacc, mybir
    import concourse.tile as tile

    f32 = mybir.dt.float32
    bf16 = mybir.dt.bfloat16
    fp8 = mybir.dt.float8e4
    DR = mybir.MatmulPerfMode.DoubleRow
    nc = bacc.Bacc("TRN2", target_bir_lowering=False, debug=False,
                   num_devices=NCORES)

    # wts[t]: slot t's row tile, [K-part, chunk, row] with frozen chunks
    # negated; cols[c]: col band c, [K-part, chunk, col]; both fp8.
    wts = nc.declare_dram_parameter("wts", [NSLOT, 128, NK, RT], fp8,
                                    isOutput=False)
    cols = nc.declare_dram_parameter("cols", [NCB, 128, NK, CBW], fp8,
                                     isOutput=False)
    cmask = nc.declare_dram_parameter("cmask", [128, NB * 16], bf16,
                                      isOutput=False)
    racc_out = nc.declare_dram_parameter("racc", [128, NB], f32, isOutput=True)
    cs_out = nc.declare_dram_parameter("cs", [16, CBW], f32, isOutput=True)

    with tile.TileContext(nc) as tc:
        with (
            tc.tile_pool(name="inp", bufs=1) as inp,
            tc.tile_pool(name="pd", bufs=6, space="PSUM") as pd,
            tc.tile_pool(name="pw", bufs=1, space="PSUM") as pw,
            tc.tile_pool(name="pcs", bufs=1, space="PSUM") as pcs,
            tc.tile_pool(name="adp", bufs=4) as adp,
            tc.tile_pool(name="outp", bufs=1) as outp,
        ):
            # Input DMAs spread over three queues (the DMA engine runs ~4
            # parallel channels): even col bands on sync, odd on gpsimd,
            # weights + mask on the otherwise-idle vector queue. The first
            # transfers are split so the first blocks' data lands early.
            wt_t = []
            for t in range(NSLOT):
                t_ = inp.tile([128, NK, RT], fp8, name=f"wt{t}", tag=f"wt{t}")
                wt_t.append(t_)
            col_t = []
            for c in range(NCB):
                t_ = inp.tile([128, NK, CBW], fp8, name=f"col{c}",
                              tag=f"col{c}")
                col_t.append(t_)
            cm_t = inp.tile([128, NB * 16], bf16, name="cm_t", tag="cm_t")

            warm_s = inp.tile([128, CBW], bf16, name="warm_s", tag="warm_s")
            nc.vector.memset(warm_s[:], 0.0)
            # weights + mask on the scalar queue (idle until its first abs)
            nc.scalar.dma_start(wt_t[0][:, :4, :], wts[0][:, :4, :])
            nc.scalar.dma_start(wt_t[0][:, 4:, :], wts[0][:, 4:, :])
            nc.scalar.dma_start(cm_t[:], cmask[:])
            # col0 in per-group pieces, col1/col2 in halves, rest whole
            for g in range(NG):
                nc.sync.dma_start(col_t[0][:, 2 * g: 2 * g + 2, :],
                                  cols[0][:, 2 * g: 2 * g + 2, :])
            nc.gpsimd.dma_start(col_t[1][:, :4, :], cols[1][:, :4, :])
            nc.gpsimd.dma_start(col_t[1][:, 4:, :], cols[1][:, 4:, :])
            nc.sync.dma_start(col_t[2][:, :4, :], cols[2][:, :4, :])
            nc.sync.dma_start(col_t[2][:, 4:, :], cols[2][:, 4:, :])
            nc.gpsimd.dma_start(wt_t[1][:], wts[1])
            for c in range(3, NCB):
                eng = nc.gpsimd if c % 2 == 1 else nc.sync
                eng.dma_start(col_t[c][:], cols[c])
                if c == 5:
                    nc.gpsimd.dma_start(wt_t[2][:], wts[2])
                if c == 9:
                    nc.gpsimd.dma_start(wt_t[3][:], wts[3])

            racc_t = outp.tile([128, NB], f32, name="racc_t", tag="racc_t")
            cs_psum = pcs.tile([16, CBW], f32, name="cs_psum", tag="cs_psum")

            # PE warm-up during the DMA wait: ramp the clock
            warm_p = pw.tile([128, CBW], f32, name="warm_p", tag="warm_p")
            for w in range(6):
                nc.tensor.matmul(warm_p[:], warm_s[:, :128], warm_s[:],
                                 start=True, stop=True)

            nmir = 0
            for b, (t, c) in enumerate(BLOCKS):
                d = pd.tile([128, CBW], f32, name=f"d_{t}_{c}", tag="d")
                for g in range(NG):
                    nc.tensor.matmul(
                        d[:],
                        wt_t[t][:, 2 * g: 2 * g + 2, :],
                        col_t[c][:, 2 * g: 2 * g + 2, :],
                        start=(g == 0),
                        stop=(g == NG - 1),
                        perf_mode=DR,
                    )
                ad = adp.tile([128, CBW], bf16, name=f"ad_{t}_{c}", tag="ad")
                nc.scalar.activation(ad[:], d[:],
                                     mybir.ActivationFunctionType.Abs)
                nc.vector.tensor_reduce(
                    out=racc_t[:, b: b + 1],
                    in_=ad[:],
                    axis=mybir.AxisListType.X,
                    op=mybir.AluOpType.add,
                )
                if c > CSTART[t]:
                    nc.tensor.matmul(
                        cs_psum[:],
                        cm_t[:, 16 * b: 16 * (b + 1)],
                        ad[:],
                        start=(nmir == 0),
                        stop=(nmir == NMIR - 1),
                    )
                    nmir += 1
                if b == 15:
                    # first 16 row-sum columns are final; overlap their DMA
                    nc.sync.dma_start(racc_out[:, :16], racc_t[:, :16])

            cs_sb = outp.tile([16, CBW], f32, name="cs_sb", tag="cs_sb")
            nc.scalar.copy(cs_sb[:], cs_psum[:])
            nc.sync.dma_start(cs_out[:], cs_sb[:])
            nc.sync.dma_start(racc_out[:, 16:], racc_t[:, 16:])

    nc.compile()
    return nc


def _get_compiled():
    global _COMPILED
    if _COMPILED is None:
        _COMPILED = _build()
    return _COMPILED


def _normalize(x):
    n = np.sqrt((x.astype(np.float64) ** 2).sum(-1, keepdims=True))
    return (x / np.maximum(n, EPS)).astype(np.float32)


def _device_rowsums(xnf, xnz):
    """xnf (M, 256), xnz (M, 768) f32 -> S (M,) row sums of |feat-frozen|."""
    global _last_bass_results
    from concourse.bass_utils import run_bass_kernel_spmd

    nc = _get_compiled()

    qf = (xnf * SCALE).astype(F8)                 # (M, 256)
    qz = (xnz * SCALE).astype(F8)                 # (M, 768)
    # K-major chunks, cols zero-padded to MC
    chunks = np.zeros((NK, 128, MC), F8)
    chunks[:2, :, :M] = np.ascontiguousarray(qf.T).reshape(2, 128, M)
    chunks[2:, :, :M] = np.ascontiguousarray(qz.T).reshape(6, 128, M)
    # cols[c, p, k, x] = chunks[k, p, CBW*c + x]
    cols_np = np.ascontiguousarray(
        chunks.reshape(NK, 128, NCB, CBW).transpose(2, 1, 0, 3))
    # weights: frozen chunks sign-flipped; rows use the MR (=3200) padding
    wneg = chunks[:, :, :MR].copy()
    wneg[2:] = (wneg[2:].view(np.uint8) ^ 0x80).view(F8)
    wall = np.ascontiguousarray(
        wneg.reshape(NK, 128, NRT, RT).transpose(2, 1, 0, 3))  # [25,128,8,128]

    in_maps = []
    for cid in range(NCORES):
        wt = np.zeros((NSLOT, 128, NK, RT), F8)
        cm = np.zeros((128, NB, 16), np.float32)
        for t in range(NSLOT):
            r = NCORES * t + cid
            if r < NRT:
                wt[t] = wall[r]
                jd = r // 2
                for b, (bt, c) in enumerate(BLOCKS):
                    if bt == t and c > jd:
                        cm[:, b, c] = 1.0
        in_maps.append({
            "wts": wt,
            "cols": cols_np,
            "cmask": np.ascontiguousarray(
                cm.reshape(128, NB * 16)).astype(BF16),
        })

    res = run_bass_kernel_spmd(nc, in_maps, list(range(NCORES)))
    _last_bass_results = res

    S = np.zeros(MC, np.float64)
    for cid in range(NCORES):
        racc = res.results[cid]["racc"].astype(np.float64)   # (128, 28)
        cs = res.results[cid]["cs"].astype(np.float64)       # (16, 256)
        for b, (t, c) in enumerate(BLOCKS):
            r = NCORES * t + cid
            if r < NRT and c >= r // 2:
                S[RT * r: RT * (r + 1)] += racc[:, b]
        S[:MC] += cs[:NCB].reshape(-1)
    return (S[:M] / (SCALE * SCALE)).astype(np.float32)


def kernel(frozen_embeddings, feature_embeddings, proto_sim, labels):
    fz = np.asarray(frozen_embeddings, dtype=np.float32).reshape(M, D)
    fn = np.asarray(feature_embeddings, dtype=np.float32).reshape(M, NF)
    ps_ = np.asarray(proto_sim, dtype=np.float32)
    lab = np.asarray(labels)

    xnf = _normalize(fn)
    xnz = _normalize(fz)

    # dense part on the 8 NeuronCores
    S = _device_rowsums(xnf, xnz)

    # prototype max/argmax and labels (host, tiny)
    psr = ps_.transpose(0, 2, 1).reshape(M, P)
    mps = psr.max(1)
    pidx = psr.argmax(1)
    ext = np.repeat(lab, N)

    # sparse ranking candidates: only same-argmax-prototype pairs can be nonzero
    cand_vals, cand_flat = [], []
    for p in np.unique(pidx):
        g = np.nonzero(pidx == p)[0]
        s = len(g)
        if s < 2:
            continue
        F = xnf[g] @ xnf[g].T
        Z = xnz[g] @ xnz[g].T
        V = (F - Z) * np.outer(mps[g], mps[g])
        iu, ju = np.triu_indices(s, 1)
        ok = ext[g][iu] != ext[g][ju]
        if ok.any():
            cand_vals.append(V[iu[ok], ju[ok]].astype(np.float64))
            cand_flat.append(g[iu[ok]].astype(np.int64) * M + g[ju[ok]])
    if cand_vals:
        vals = np.concatenate(cand_vals)
        flats = np.concatenate(cand_flat)
    else:
        vals = np.zeros(0)
        flats = np.zeros(0, np.int64)

    # top-5 with lax.top_k tie semantics (desc value, then asc flat index);
    # entries not in the candidate set are exact zeros in the ranking matrix.
    order = np.lexsort((flats, -vals))
    pos = [f for f in order if vals[f] > 0][:K_]
    sel_flats = [int(flats[i]) for i in pos]
    if len(sel_flats) < K_:
        nonzero = set(int(f) for v, f in zip(vals, flats) if v != 0.0)
        f = 0
        while len(sel_flats) < K_:
            if f not in nonzero:
                sel_flats.append(f)
            f += 1
    sel_flats = np.asarray(sel_flats, np.int64)
    rows = sel_flats // M
    cols_sel = sel_flats % M

    out = GAMMA * (S[rows].sum(dtype=np.float64)
                   + S[cols_sel].sum(dtype=np.float64)) / (2 * K_ * M)
    return np.asarray(np.float32(out))


# revision 11
# speedup vs baseline: 1.1410x; 1.1410x over previous
"""Trainium2 kernel for nn_ConservationOfFeatureSimilarity.

Math (see reference): with xn = row-normalized feature embeddings (M, 256) and
zn = row-normalized frozen embeddings (M, 768), M = B*N = 3136:

  feat_sim  = xn @ xn.T        (M, M)
  frozen_sim= zn @ zn.T        (M, M)
  ranking   = triu+ * (feat-frozen) * [cls_i != cls_j] * [pidx_i == pidx_j] * mps_i*mps_j
  top5      = top_k(ranking.flat, 5);  sel rows/cols
  out       = mean |feat_sim[sel] - frozen_sim[sel]|  over (5, 2, M)
            = (sum over the 10 selected row indices of S[r]) / (10*M)
  where S_i = sum_j |feat_sim[i,j] - frozen_sim[i,j]|.

Device (8 NeuronCores): the dense O(M^2 * K) part — S row sums — runs as fp8
(e4m3) DoubleRow matmuls: embeddings are scaled by 16, quantized to fp8, and
diff tiles accumulate in PSUM over 4 DoubleRow groups (256-deep contraction
each; frozen chunks sign-flipped on the weights side so one gram matrix is
subtracted). |diff| is symmetric, so only blocks at-or-right-of the diagonal
of a (25 row-tile x 13 col-block) grid are computed: rows padded to
3200 = 25*128, cols to 3328 = 13*256. Core c owns row tiles {8t+c: t=0..3};
slot t computes col blocks c >= 4t (28 blocks/core; 13+9+5+1). Each block
contributes DVE row sums, and strictly-upper blocks contribute mirrored
column sums via a masked ones-matmul (the per-core 0/1 mask is data, not
code). The host drops the few below-diagonal row-sum partials.

Host: normalization/transposes, fp8 quantization, prototype argmax, the top-5
search (ranking is nonzero only for same-argmax-prototype pairs: ~25K of the
9.8M pairs, so it is evaluated sparsely in numpy), and the final combine.
"""

import sys

if "/opt/trn_rl_repo" not in sys.path:
    sys.path.insert(0, "/opt/trn_rl_repo")

import numpy as np
import ml_dtypes

BF16 = ml_dtypes.bfloat16
F8 = ml_dtypes.float8_e4m3

B, N, D, NF, P = 16, 196, 768, 256, 200
M = B * N                      # 3136
NCORES = 8
RT = 128                       # row tile height
NRT = 25                       # row tiles (rows padded to 3200)
MR = RT * NRT                  # 3200
CBW = 256                      # col block width
NCB = 13                       # col blocks (cols padded to 3328)
MC = CBW * NCB                 # 3328
NK = 8                         # 128-deep K chunks: 2 feat + 6 frozen
NG = 4                         # DoubleRow groups (256-deep each)
NSLOT = 4                      # row tiles per core (slot t -> tile 8t+c)
CSTART = (0, 4, 8, 12)         # first col block computed by slot t
SCALE = 16.0                   # fp8 quantization scale
K_ = 5
GAMMA = 1.0
EPS = 1e-8

# program-order block list: col-major so compute overlaps the col-band DMAs
BLOCKS = [(t, c) for c in range(NCB) for t in range(NSLOT) if c >= CSTART[t]]
NB = len(BLOCKS)               # 28
NMIR = sum(1 for t, c in BLOCKS if c > CSTART[t])  # 24 mirrored blocks

_COMPILED = None
_last_bass_results = None


def _build():
    from concourse import bacc, mybir
    import concourse.tile as tile

    f32 = mybir.dt.float32
    bf16 = mybir.dt.bfloat16
    fp8 = mybir.dt.float8e4
    DR = mybir.MatmulPerfMode.DoubleRow
    nc = bacc.Bacc("TRN2", target_bir_lowering=False, debug=False,
                   num_devices=NCORES)

    # wts[t]: slot t's row tile, [K-part, chunk, row] with frozen chunks
    # negated; cols[c]: col band c, [K-part, chunk, col]; both fp8.
    wts = nc.declare_dram_parameter("wts", [NSLOT, 128, NK, RT], fp8,
                                    isOutput=False)
    cols = nc.declare_dram_parameter("cols", [NCB, 128, NK, CBW], fp8,
                                     isOutput=False)
    cmask = nc.declare_dram_parameter("cmask", [128, NB * 16], bf16,
                                      isOutput=False)
    racc_out = nc.declare_dram_parameter("racc", [128, NB], f32, isOutput=True)
    cs_out = nc.declare_dram_parameter("cs", [16, CBW], f32, isOutput=True)

    with tile.TileContext(nc) as tc:
        with (
            tc.tile_pool(name="inp", bufs=1) as inp,
            tc.tile_pool(name="pd", bufs=7, space="PSUM") as pd,
            tc.tile_pool(name="pcs", bufs=1, space="PSUM") as pcs,
            tc.tile_pool(name="adp", bufs=4) as adp,
            tc.tile_pool(name="outp", bufs=1) as outp,
        ):
            # Input DMAs on three queues (each sustains ~220 GB/s): col
            # bands alternate sync/gpsimd in compute order; weights + mask
            # ride the scalar queue, which is otherwise idle early.
            wt_t = []
            for t in range(NSLOT):
                t_ = inp.tile([128, NK, RT], fp8, name=f"wt{t}", tag=f"wt{t}")
                wt_t.append(t_)
            col_t = []
            for c in range(NCB):
                t_ = inp.tile([128, NK, CBW], fp8, name=f"col{c}",
                              tag=f"col{c}")
                col_t.append(t_)
            cm_t = inp.tile([128, NB * 16], bf16, name="cm_t", tag="cm_t")

            warm_s = inp.tile([128, CBW], bf16, name="warm_s", tag="warm_s")
            nc.vector.memset(warm_s[:], 0.0)
            nc.scalar.dma_start(wt_t[0][:], wts[0])
            nc.scalar.dma_start(cm_t[:], cmask[:])
            for t in range(1, NSLOT):
                nc.scalar.dma_start(wt_t[t][:], wts[t])
            for c in range(NCB):
                eng = nc.gpsimd if c % 2 == 1 else nc.sync
                eng.dma_start(col_t[c][:], cols[c])

            racc_t = outp.tile([128, NB], f32, name="racc_t", tag="racc_t")
            cs_psum = pcs.tile([16, CBW], f32, name="cs_psum", tag="cs_psum")

            # PE warm-up during the DMA wait: ramp the clock
            for w in range(6):
                warm_p = pd.tile([128, CBW], f32, name=f"warm{w}", tag="d")
                nc.tensor.matmul(warm_p[:], warm_s[:, :128], warm_s[:],
                                 start=True, stop=True)

            nmir = 0
            for b, (t, c) in enumerate(BLOCKS):
                d = pd.tile([128, CBW], f32, name=f"d_{t}_{c}", tag="d")
                for g in range(NG):
                    nc.tensor.matmul(
                        d[:],
                        wt_t[t][:, 2 * g: 2 * g + 2, :],
                        col_t[c][:, 2 * g: 2 * g + 2, :],
                        start=(g == 0),
                        stop=(g == NG - 1),
                        perf_mode=DR,
                    )
                ad = adp.tile([128, CBW], bf16, name=f"ad_{t}_{c}", tag="ad")
                # |d| and its row-sum fused; ~1/3 of blocks go to the DVE
                # (2 ops there: negate, then max-reduce) to unload Scalar
                if True:
                    nc.scalar.activation(ad[:], d[:],
                                         mybir.ActivationFunctionType.Abs,
                                         accum_out=racc_t[:, b: b + 1])
                elif False:
                    nd = adp.tile([128, CBW], bf16, name=f"nd_{t}_{c}",
                                  tag="ad")
                    nc.vector.tensor_scalar(nd[:], d[:], -1.0, None,
                                            op0=mybir.AluOpType.mult)
                    nc.vector.tensor_tensor_reduce(
                        ad[:], d[:], nd[:], scale=1.0, scalar=0.0,
                        op0=mybir.AluOpType.max,
                        op1=mybir.AluOpType.add,
                        accum_out=racc_t[:, b: b + 1])
                if c > CSTART[t]:
                    nc.tensor.matmul(
                        cs_psum[:],
                        cm_t[:, 16 * b: 16 * (b + 1)],
                        ad[:],
                        start=(nmir == 0),
                        stop=(nmir == NMIR - 1),
                    )
                    nmir += 1
                if b == 15:
                    # first 16 row-sum columns are final; overlap their DMA
                    nc.sync.dma_start(racc_out[:, :16], racc_t[:, :16])

            cs_sb = outp.tile([16, CBW], f32, name="cs_sb", tag="cs_sb")
            nc.scalar.copy(cs_sb[:], cs_psum[:])
            nc.sync.dma_start(cs_out[:], cs_sb[:])
            nc.sync.dma_start(racc_out[:, 16:], racc_t[:, 16:])

    nc.compile()
    return nc


def _get_compiled():
    global _COMPILED
    if _COMPILED is None:
        _COMPILED = _build()
    return _COMPILED


def _normalize(x):
    n = np.sqrt((x.astype(np.float64) ** 2).sum(-1, keepdims=True))
    return (x / np.maximum(n, EPS)).astype(np.float32)


def _device_rowsums(xnf, xnz):
    """xnf (M, 256), xnz (M, 768) f32 -> S (M,) row sums of |feat-frozen|."""
    global _last_bass_results
    from concourse.bass_utils import run_bass_kernel_spmd

    nc = _get_compiled()

    qf = (xnf * SCALE).astype(F8)                 # (M, 256)
    qz = (xnz * SCALE).astype(F8)                 # (M, 768)
    # K-major chunks, cols zero-padded to MC
    chunks = np.zeros((NK, 128, MC), F8)
    chunks[:2, :, :M] = np.ascontiguousarray(qf.T).reshape(2, 128, M)
    chunks[2:, :, :M] = np.ascontiguousarray(qz.T).reshape(6, 128, M)
    # cols[c, p, k, x] = chunks[k, p, CBW*c + x]
    cols_np = np.ascontiguousarray(
        chunks.reshape(NK, 128, NCB, CBW).transpose(2, 1, 0, 3))
    # weights: frozen chunks sign-flipped; rows use the MR (=3200) padding
    wneg = chunks[:, :, :MR].copy()
    wneg[2:] = (wneg[2:].view(np.uint8) ^ 0x80).view(F8)
    wall = np.ascontiguousarray(
        wneg.reshape(NK, 128, NRT, RT).transpose(2, 1, 0, 3))  # [25,128,8,128]

    in_maps = []
    for cid in range(NCORES):
        wt = np.zeros((NSLOT, 128, NK, RT), F8)
        cm = np.zeros((128, NB, 16), np.float32)
        for t in range(NSLOT):
            r = NCORES * t + cid
            if r < NRT:
                wt[t] = wall[r]
                jd = r // 2
                for b, (bt, c) in enumerate(BLOCKS):
                    if bt == t and c > jd:
                        cm[:, b, c] = 1.0
        in_maps.append({
            "wts": wt,
            "cols": cols_np,
            "cmask": np.ascontiguousarray(
                cm.reshape(128, NB * 16)).astype(BF16),
        })

    res = run_bass_kernel_spmd(nc, in_maps, list(range(NCORES)))
    _last_bass_results = res

    S = np.zeros(MC, np.float64)
    for cid in range(NCORES):
        racc = res.results[cid]["racc"].astype(np.float64)   # (128, 28)
        cs = res.results[cid]["cs"].astype(np.float64)       # (16, 256)
        for b, (t, c) in enumerate(BLOCKS):
            r = NCORES * t + cid
            if r < NRT and c >= r // 2:
                S[RT * r: RT * (r + 1)] += racc[:, b]
        S[:MC] += cs[:NCB].reshape(-1)
    return (S[:M] / (SCALE * SCALE)).astype(np.float32)


def kernel(frozen_embeddings, feature_embeddings, proto_sim, labels):
    fz = np.asarray(frozen_embeddings, dtype=np.float32).reshape(M, D)
    fn = np.asarray(feature_embeddings, dtype=np.float32).reshape(M, NF)
    ps_ = np.asarray(proto_sim, dtype=np.float32)
    lab = np.asarray(labels)

    xnf = _normalize(fn)
    xnz = _normalize(fz)

    # dense part on the 8 NeuronCores
    S = _device_rowsums(xnf, xnz)

    # prototype max/argmax and labels (host, tiny)
    psr = ps_.transpose(0, 2, 1).reshape(M, P)
    mps = psr.max(1)
    pidx = psr.argmax(1)
    ext = np.repeat(lab, N)

    # sparse ranking candidates: only same-argmax-prototype pairs can be nonzero
    cand_vals, cand_flat = [], []
    for p in np.unique(pidx):
        g = np.nonzero(pidx == p)[0]
        s = len(g)
        if s < 2:
            continue
        F = xnf[g] @ xnf[g].T
        Z = xnz[g] @ xnz[g].T
        V = (F - Z) * np.outer(mps[g], mps[g])
        iu, ju = np.triu_indices(s, 1)
        ok = ext[g][iu] != ext[g][ju]
        if ok.any():
            cand_vals.append(V[iu[ok], ju[ok]].astype(np.float64))
            cand_flat.append(g[iu[ok]].astype(np.int64) * M + g[ju[ok]])
    if cand_vals:
        vals = np.concatenate(cand_vals)
        flats = np.concatenate(cand_flat)
    else:
        vals = np.zeros(0)
        flats = np.zeros(0, np.int64)

    # top-5 with lax.top_k tie semantics (desc value, then asc flat index);
    # entries not in the candidate set are exact zeros in the ranking matrix.
    order = np.lexsort((flats, -vals))
    pos = [f for f in order if vals[f] > 0][:K_]
    sel_flats = [int(flats[i]) for i in pos]
    if len(sel_flats) < K_:
        nonzero = set(int(f) for v, f in zip(vals, flats) if v != 0.0)
        f = 0
        while len(sel_flats) < K_:
            if f not in nonzero:
                sel_flats.append(f)
            f += 1
    sel_flats = np.asarray(sel_flats, np.int64)
    rows = sel_flats // M
    cols_sel = sel_flats % M

    out = GAMMA * (S[rows].sum(dtype=np.float64)
                   + S[cols_sel].sum(dtype=np.float64)) / (2 * K_ * M)
    return np.asarray(np.float32(out))
